# revision 38
# baseline (speedup 1.0000x reference)
"""Causal self-attention (RoPE) Trainium2 Bass kernel.

Sharding: 8 cores = 2 (batch) x 4 (head groups). Each core computes one batch
element b and 4 of the 16 heads end-to-end (QKV projection -> RoPE -> causal
attention -> c_proj rows), producing a partial [T, C] output; the host sums
the 4 partials per batch element (the "all-reduce" of the row-sharded c_proj).

Dataflow is feature-major ("transposed"): qkv is computed as [feat, token] so
the scores matmul (contract over head dim) and the AV matmul (contract over
keys) chain without any on-chip transposes. V is projected separately in
[token, feat] layout (it is the AV stationary operand). Scores are computed
transposed (S^T[tk, tq]) so softmax sums land on a matmul ones-column.

RoPE trick: W_attn's Q/K columns are reordered on the host so the projection
emits Q_lo = [h0 d0:32 | h1 d0:32 | ...] and Q_hi = [h0 d32:64 | ...] as two
128-partition blocks; rotate_half is then two full-width [128, T] tensor ops
instead of per-head 32-row slices.
"""

import os
import sys
import numpy as np

N_CORES = 8
B, T, C = 2, 2048, 1024
H = 16
HD = 64
HPC = 4            # heads per core
NT = 4             # token tiles of 512
TQ = 512           # tq tile size
KC = C // 128      # contraction chunks for qkv projection

_PROGRAM_CACHE = {}


def _build_program(has_battn: bool, has_bproj: bool, p_bf16: bool,
                   debug_taps: bool = False):
    import concourse.bass as bass
    import concourse.mybir as mybir
    import concourse.bacc as bacc
    import concourse.tile as tile

    F32 = mybir.dt.float32
    F32R = mybir.dt.float32r
    PDT = mybir.dt.bfloat16 if p_bf16 else F32R

    nc = bacc.Bacc("TRN2", target_bir_lowering=False, debug=False,
                   num_devices=N_CORES)

    dbg = {}
    if debug_taps:
        for name, shape in [("dbg_qlo", [128, T]), ("dbg_khi", [128, T]),
                            ("dbg_vaug", [128, 260]), ("dbg_pfull", [128, 512]),
                            ("dbg_pdiag", [128, 512]), ("dbg_av", [65, 512]),
                            ("dbg_rec", [1, 512]), ("dbg_y0", [128, T])]:
            dbg[name] = nc.dram_tensor(name, shape, F32,
                                       kind="ExternalOutput").ap()

    xT = nc.dram_tensor("xT", [C, T], PDT, kind="ExternalInput").ap()
    wqkv = nc.dram_tensor("wqkv", [C, 768], PDT, kind="ExternalInput").ap()
    cos4 = nc.dram_tensor("cos4", [128, T], F32, kind="ExternalInput").ap()
    sin4 = nc.dram_tensor("sin4", [128, T], F32, kind="ExternalInput").ap()
    wp = nc.dram_tensor("wp", [2 * 128, C], F32R, kind="ExternalInput").ap()
    battn = (nc.dram_tensor("battn", [1, 768], PDT, kind="ExternalInput").ap()
             if has_battn else None)
    bproj = (nc.dram_tensor("bproj", [1, C], F32R, kind="ExternalInput").ap()
             if has_bproj else None)
    out = nc.dram_tensor("out", [T, C], PDT, kind="ExternalOutput").ap()
    # DRAM scratch for the 1/sumexp partition-broadcast bounce
    rec_d = nc.dram_tensor("rec_d", [16, 512], F32)

    Exp = mybir.ActivationFunctionType.Exp
    scale = 1.0 / float(np.sqrt(HD))

    with tile.TileContext(nc) as tc:
        with (
            tc.tile_pool(name="const", bufs=1) as const,
            tc.tile_pool(name="xp", bufs=3) as xp,
            tc.tile_pool(name="qk", bufs=1) as qkp,
            tc.tile_pool(name="vaug", bufs=1) as vaugp,
            tc.tile_pool(name="tmp", bufs=3) as tmp,
            tc.tile_pool(name="pp", bufs=4) as pp,
            tc.tile_pool(name="rp", bufs=4) as rp,
            tc.tile_pool(name="yrp", bufs=4) as yrp,
            tc.tile_pool(name="yp", bufs=1) as yp,
            tc.tile_pool(name="op", bufs=2) as op,
            tc.tile_pool(name="ps1", bufs=4, space="PSUM") as ps1,
            tc.tile_pool(name="ps2", bufs=2, space="PSUM") as ps2,
        ):
            # ---- constants — interleaved with the first x tile so the
            # first qkv matmul can start after ~2 chunks
            w_sb = const.tile([128, KC, 768], PDT, tag="w")
            xt_view0 = xT.rearrange("(kc p) t -> p kc t", p=128)
            xt0 = xp.tile([128, KC, TQ], PDT, tag="xt", name="xt_0")
            for kc in range(KC):
                nc.sync.dma_start(
                    w_sb[:, kc, :],
                    wqkv.rearrange("(kc p) f -> p kc f", p=128)[:, kc, :])
                nc.sync.dma_start(xt0[:, kc, :], xt_view0[:, kc, 0:TQ])
            cos_sb = const.tile([128, T], F32, tag="cos")
            nc.sync.dma_start(cos_sb[:], cos4[:])
            sin_sb = const.tile([128, T], F32, tag="sin")
            nc.sync.dma_start(sin_sb[:], sin4[:])
            wp_sb = const.tile([128, 2, C], F32R, tag="wp")
            nc.sync.dma_start(wp_sb[:], wp.rearrange("(kb p) f -> p kb f", p=128))
            if has_battn or has_bproj:
                ones_f = const.tile([1, 512], F32, tag="ones_f")
                nc.vector.memset(ones_f[:], 1.0)
                # memset can't write float32r; round-copy from f32 staging
                ones_r = const.tile([1, 512], F32R, tag="ones_r")
                nc.vector.tensor_copy(ones_r[:], ones_f[:])
                ones_b = const.tile([1, 512], PDT, tag="ones_b")
                nc.vector.tensor_copy(ones_b[:], ones_f[:])
            if has_battn:
                battn_sb = const.tile([1, 768], PDT, tag="battn")
                nc.sync.dma_start(battn_sb[:], battn[:])
            if has_bproj:
                bproj_sb = const.tile([1, C], F32R, tag="bproj")
                nc.sync.dma_start(bproj_sb[:], bproj[:])

            # persistent activations
            q_lo = qkp.tile([128, T], F32R, tag="qlo")
            q_hi = qkp.tile([128, T], F32R, tag="qhi")
            k_lo = qkp.tile([128, T], F32R, tag="klo")
            k_hi = qkp.tile([128, T], F32R, tag="khi")
            v_aug = vaugp.tile([128, 16, HPC * 65], PDT, tag="vaug")
            y0 = yp.tile([128, T], F32R, tag="y0")
            y1 = yp.tile([128, T], F32R, tag="y1")

            # ones columns of v_aug (col 64 of each head's 65-wide slot)
            for tb in range(16):
                va = v_aug[:, tb, :].rearrange("p (h c) -> p h c", c=65)
                nc.vector.memset(va[:, :, 64:65], 1.0)

            xt_view = xT.rearrange("(kc p) t -> p kc t", p=128)

            # prefetch remaining x tiles so qkv matmuls never wait on DMA
            xts = [xt0]
            for n in range(1, NT):
                tsl = bass.ts(n, TQ)
                xt = xp.tile([128, KC, TQ], PDT, tag="xt", name=f"xt_{n}")
                for kc in range(KC):
                    nc.sync.dma_start(xt[:, kc, :], xt_view[:, kc, tsl])
                xts.append(xt)

            # qkv projection + rope + v staging for token tile n
            def qkv_tile(n):
                tsl = bass.ts(n, TQ)
                xt = xts[n]

                # q_lo|q_hi pair then k_lo|k_hi pair, each in a 2-bank psum
                for (pair, d_lo, d_hi) in ((0, q_lo, q_hi), (1, k_lo, k_hi)):
                    pq = ps2.tile([128, 2 * TQ], F32, tag="ps2",
                                  name=f"qk_{n}_{pair}")
                    for half in range(2):
                        fb = 2 * pair + half
                        dst = pq[:, half * TQ:(half + 1) * TQ]
                        for kc in range(KC):
                            nc.tensor.matmul(
                                dst,
                                w_sb[:, kc, bass.ts(fb, 128)],
                                xt[:, kc, :],
                                start=(kc == 0),
                                stop=(kc == KC - 1 and not has_battn),
                            )
                        if has_battn:
                            nc.tensor.matmul(
                                dst,
                                battn_sb[0:1, bass.ts(fb, 128)],
                                ones_b[0:1, :TQ],
                                start=False, stop=True,
                            )
                    # rope fused into psum eviction:
                    #   out_lo = lo*cos - hi*sin ; out_hi = hi*cos + lo*sin
                    cs = cos_sb[:, tsl]
                    sn = sin_sb[:, tsl]
                    p_lo = pq[:, 0:TQ]
                    p_hi = pq[:, TQ:2 * TQ]
                    t1 = tmp.tile([128, TQ], F32, tag="t")
                    nc.vector.tensor_mul(t1[:], p_lo, cs)
                    t2 = tmp.tile([128, TQ], F32, tag="t")
                    nc.vector.tensor_mul(t2[:], p_hi, sn)
                    nc.vector.tensor_sub(d_lo[:, tsl], t1[:], t2[:])
                    t3 = tmp.tile([128, TQ], F32, tag="t")
                    nc.vector.tensor_mul(t3[:], p_hi, cs)
                    t4 = tmp.tile([128, TQ], F32, tag="t")
                    nc.vector.tensor_mul(t4[:], p_lo, sn)
                    nc.vector.tensor_add(d_hi[:, tsl], t3[:], t4[:])

                # v in [token, feat] layout: lhsT = xT chunk, rhs = w_v
                for tb in range(4):
                    blk = 4 * n + tb
                    pv = ps1.tile([128, 512], F32, tag="ps1",
                                  name=f"pv_{n}_{tb}")
                    for kc in range(KC):
                        nc.tensor.matmul(
                            pv[:, 0:256],
                            xt[:, kc, bass.ts(tb, 128)],
                            w_sb[:, kc, 512:768],
                            start=(kc == 0),
                            stop=(kc == KC - 1 and not has_battn),
                        )
                    if has_battn:
                        nc.tensor.matmul(
                            pv[:, 0:256], ones_b[0:1, 0:128],
                            battn_sb[0:1, 512:768],
                            start=False, stop=True,
                        )
                    va = v_aug[:, blk, :].rearrange("p (h c) -> p h c", c=65)
                    pv_h = pv[:, 0:256].rearrange("p (h d) -> p h d", d=64)
                    nc.vector.tensor_copy(va[:, :, 0:64], pv_h[:])

            # attention + c_proj for query tile i (needs qkv tiles 0..i)
            def attn_tile(i, deferred):
                tq_sl = bass.ts(i, TQ)
                av_ps = [ps1.tile([65, TQ], F32, tag="ps1", name=f"av_{i}_{h}")
                         for h in range(HPC)]
                n_k = 4 * i + 4
                for kb in range(n_k):
                    ksl = bass.ts(kb, 128)
                    j = kb - 4 * i
                    skip = 128 * j if j > 0 else 0   # fully-masked lead cols
                    W_v = TQ - skip                  # valid width per half
                    for pair in range(2):
                        s2 = ps2.tile([128, 2 * TQ], F32, tag="ps2",
                                      name=f"s_{i}_{kb}_{pair}")
                        for half in range(2):
                            h = 2 * pair + half
                            hs = slice(32 * h, 32 * h + 32)
                            dst = s2[:, half * TQ + skip:(half + 1) * TQ]
                            qsl = bass.ds(i * TQ + skip, W_v)
                            nc.tensor.matmul(dst, k_lo[hs, ksl],
                                             q_lo[hs, qsl],
                                             start=True, stop=False,
                                             tile_position=(32 * h, 0))
                            nc.tensor.matmul(dst, k_hi[hs, ksl],
                                             q_hi[hs, qsl],
                                             start=False, stop=True,
                                             tile_position=(32 * h, 0))
                        p_t = pp.tile([128, 2 * TQ], PDT, tag="p")
                        s2_v = s2[:].rearrange("p (g t) -> p g t", g=2)
                        pt_v = p_t[:].rearrange("p (g t) -> p g t", g=2)
                        nc.scalar.activation(pt_v[:, :, skip:TQ],
                                             s2_v[:, :, skip:TQ],
                                             Exp, scale=scale)
                        if skip:
                            nc.vector.memset(pt_v[:, :, 0:skip], 0.0)
                        if j >= 0:
                            # 128-wide causal triangle (keep local col >= row)
                            for half in range(2):
                                off = half * TQ + skip
                                nc.gpsimd.affine_select(
                                    out=p_t[:, off:off + 128],
                                    in_=p_t[:, off:off + 128],
                                    compare_op=mybir.AluOpType.is_ge,
                                    fill=0.0,
                                    base=0,
                                    pattern=[[1, 128]],
                                    channel_multiplier=-1,
                                )
                        if debug_taps and i == 1 and pair == 0 and kb in (2, 5):
                            nm = "dbg_pfull" if kb == 2 else "dbg_pdiag"
                            dp = tmp.tile([128, 512], F32, tag="dbgp",
                                          name=f"dbgp_{kb}")
                            nc.vector.tensor_copy(dp[:], p_t[:, TQ:2 * TQ])
                            nc.sync.dma_start(dbg[nm][:], dp[:])
                        for half in range(2):
                            h = 2 * pair + half
                            nc.tensor.matmul(
                                av_ps[h][:],
                                v_aug[:, kb, bass.ts(h, 65)],
                                p_t[:, half * TQ:(half + 1) * TQ],
                                start=(kb == 0),
                                stop=(kb == n_k - 1),
                            )
                    if kb == 0:
                        for fn in deferred:
                            fn()
                        deferred.clear()

                # evict raw av immediately (frees the psum accumulators),
                # then normalize off the critical path: 1/sum via approx
                # reciprocal, partition-broadcast via a DRAM-bounce DMA
                for h in range(HPC):
                    yr = yrp.tile([65, TQ], F32, tag="yr")
                    nc.any.tensor_copy(yr[:], av_ps[h][:])
                    # reciprocal_approx_fast needs a base-partition-0 SBUF src
                    se = rp.tile([1, TQ], F32, tag="se")
                    nc.any.tensor_copy(se[:], yr[64:65, :])
                    rec = rp.tile([1, TQ], F32, tag="r")
                    nc.vector.reciprocal_approx_fast(rec[:], se[:])
                    if debug_taps and i == 1 and h == 1:
                        nc.sync.dma_start(dbg["dbg_av"][:], yr[:])
                        nc.sync.dma_start(dbg["dbg_rec"][:], rec[:])
                    slot = rec_d[4 * i + h:4 * i + h + 1, :]
                    nc.sync.dma_start(slot, rec[:])
                    bc_sb = tmp.tile([64, TQ], F32, tag="bc")
                    bcast_src = bass.AP(
                        tensor=slot.tensor, offset=slot.offset,
                        ap=[[0, 64], [1, TQ]])
                    nc.sync.dma_start(bc_sb[:], bcast_src)
                    y_t = y0 if h < 2 else y1
                    y_sl = y_t[(h % 2) * 64:(h % 2) * 64 + 64, tq_sl]
                    nc.vector.tensor_mul(y_sl, yr[0:64, :], bc_sb[:])

                # c_proj for the 4 finished token blocks; deferred so it
                # fills PE gaps during the NEXT tile instead of stalling
                def cproj():
                    for m in range(4 * i, 4 * i + 4):
                        msl = bass.ts(m, 128)
                        for n2 in range(2):
                            nsl = bass.ts(n2, 512)
                            po = ps1.tile([128, 512], F32, tag="ps1",
                                          name=f"po_{m}_{n2}")
                            nc.tensor.matmul(po[:], y0[:, msl],
                                             wp_sb[:, 0, nsl],
                                             start=True, stop=False)
                            nc.tensor.matmul(po[:], y1[:, msl],
                                             wp_sb[:, 1, nsl],
                                             start=False, stop=not has_bproj)
                            if has_bproj:
                                nc.tensor.matmul(
                                    po[:], ones_r[0:1, 0:128],
                                    bproj_sb[0:1, nsl],
                                    start=False, stop=True,
                                )
                            o_t = op.tile([128, 512], PDT, tag="o")
                            nc.any.tensor_copy(o_t[:], po[:])
                            nc.sync.dma_start(out[msl, nsl], o_t[:])
                return cproj

            # interleave so rope (DVE) overlaps attention (ACT/PE)
            deferred = []
            for n in range(NT):
                qkv_tile(n)
                deferred = [attn_tile(n, deferred)]
            deferred[0]()

            if debug_taps:
                nc.sync.dma_start(dbg["dbg_qlo"][:], q_lo[:].bitcast(F32))
                nc.sync.dma_start(dbg["dbg_khi"][:], k_hi[:].bitcast(F32))
                dv = tmp.tile([128, 260], F32, tag="dbgv")
                nc.vector.tensor_copy(dv[:], v_aug[:, 3, :])
                nc.sync.dma_start(dbg["dbg_vaug"][:], dv[:])
                nc.sync.dma_start(dbg["dbg_y0"][:], y0[:].bitcast(F32))

    nc.finalize()
    return nc


def _get_program(has_battn, has_bproj, p_bf16=True):
    key = (has_battn, has_bproj, p_bf16)
    if key not in _PROGRAM_CACHE:
        _PROGRAM_CACHE[key] = _build_program(*key)
    return _PROGRAM_CACHE[key]


def _rope_tables_np():
    inv_freq = (1.0 / (10000.0 ** (np.arange(0, HD, 2, dtype=np.float32) / HD)))
    t = np.arange(T, dtype=np.float32)
    freqs = np.outer(t, inv_freq).astype(np.float32)      # [T, 32]
    cos4 = np.tile(np.cos(freqs).T, (4, 1))               # [128, T]
    sin4 = np.tile(np.sin(freqs).T, (4, 1))
    return np.ascontiguousarray(cos4), np.ascontiguousarray(sin4)


def _install_trace_shim():
    """Optional: lets run_bass_kernel_spmd(trace=True) capture NTFF profiles."""
    import contextlib
    import ctypes
    import types

    so = "/opt/axon/libaxon_pjrt.so"
    if not os.path.exists(so) or "antenv.axon_hooks" in sys.modules:
        return
    try:
        lib = ctypes.CDLL(so)
        if not hasattr(lib, "axon_start_nrt_profile"):
            return
        lib.axon_start_nrt_profile.argtypes = [ctypes.POINTER(ctypes.c_int64),
                                               ctypes.c_size_t]
        lib.axon_start_nrt_profile.restype = ctypes.c_int64
        lib.axon_stop_nrt_profile.argtypes = [ctypes.c_char_p]
        lib.axon_stop_nrt_profile.restype = ctypes.c_int64

        @contextlib.contextmanager
        def _hook(output_dir, device_ids):
            import jax
            jax.devices()
            if device_ids:
                ids = (ctypes.c_int64 * len(device_ids))(*device_ids)
                rc = lib.axon_start_nrt_profile(ids, len(device_ids))
            else:
                rc = lib.axon_start_nrt_profile(None, 0)
            if rc != 0:
                raise RuntimeError(f"axon_start_nrt_profile rc={rc}")
            try:
                yield
            finally:
                n = lib.axon_stop_nrt_profile(str(output_dir).encode())
                print(f"profile: {n} file(s) written to {output_dir}",
                      file=sys.stderr)

        mod = types.ModuleType("antenv.axon_hooks")
        mod.get_axon_ntff_profile_hook = lambda: _hook
        mod.set_axon_ntff_profile_hook = lambda h: None
        sys.modules["antenv.axon_hooks"] = mod
    except Exception:
        pass


def kernel(x, W_attn, b_attn, W_proj, b_proj):
    from concourse.bass_utils import run_bass_kernel_spmd

    x = np.asarray(x, dtype=np.float32)
    W_attn = np.asarray(W_attn, dtype=np.float32)
    b_attn = np.asarray(b_attn, dtype=np.float32)
    W_proj = np.asarray(W_proj, dtype=np.float32)
    b_proj = np.asarray(b_proj, dtype=np.float32)

    has_battn = bool(np.any(b_attn))
    has_bproj = bool(np.any(b_proj))
    nc = _get_program(has_battn, has_bproj)

    cos4, sin4 = _rope_tables_np()
    dd32 = np.arange(32)
    dd64 = np.arange(64)

    in_maps = []
    for c in range(N_CORES):
        b = c // 4
        g = c % 4
        hs = 4 * g + np.arange(HPC)
        qlo = (hs[:, None] * HD + dd32[None, :]).ravel()
        qhi = qlo + 32
        cols = np.concatenate([qlo, qhi, 1024 + qlo, 1024 + qhi,
                               2048 + (hs[:, None] * HD + dd64[None, :]).ravel()])
        rows = (hs[:, None] * HD + dd64[None, :]).ravel()
        import ml_dtypes
        m = {
            "xT": np.ascontiguousarray(x[b].T).astype(ml_dtypes.bfloat16),
            "wqkv": np.ascontiguousarray(W_attn[:, cols]).astype(ml_dtypes.bfloat16),
            "cos4": cos4,
            "sin4": sin4,
            "wp": np.ascontiguousarray(W_proj[rows, :]),
        }
        if has_battn:
            m["battn"] = np.ascontiguousarray(
                b_attn[cols][None, :]).astype(ml_dtypes.bfloat16)
        if has_bproj:
            # only one core per batch group contributes the bias (host sums 4)
            bp = b_proj if g == 0 else np.zeros_like(b_proj)
            m["bproj"] = np.ascontiguousarray(bp[None, :])
        in_maps.append(m)

    trace_dir = os.environ.get("BASSK_TRACE")
    kwargs = {}
    if trace_dir:
        _install_trace_shim()
        kwargs = {"trace": True, "tmpdir": trace_dir,
                  "trace_cores": [0], "stitch_traces": False}

    res = run_bass_kernel_spmd(nc, in_maps, core_ids=list(range(N_CORES)),
                               **kwargs)
    if trace_dir:
        kernel._last_result = res

    out = np.zeros((B, T, C), dtype=np.float32)
    for c in range(N_CORES):
        out[c // 4] += res.results[c]["out"].astype(np.float32)
    return out


# revision 39
# speedup vs baseline: 1.0499x; 1.0499x over previous
"""Causal self-attention (RoPE) Trainium2 Bass kernel.

Sharding: 8 cores = 2 (batch) x 4 (head groups). Each core computes one batch
element b and 4 of the 16 heads end-to-end (QKV projection -> RoPE -> causal
attention -> c_proj rows), producing a partial [T, C] output; the host sums
the 4 partials per batch element (the "all-reduce" of the row-sharded c_proj).

Dataflow is feature-major ("transposed"): qkv is computed as [feat, token] so
the scores matmul (contract over head dim) and the AV matmul (contract over
keys) chain without any on-chip transposes. V is projected separately in
[token, feat] layout (it is the AV stationary operand). Scores are computed
transposed (S^T[tk, tq]) so softmax sums land on a matmul ones-column.

RoPE trick: W_attn's Q/K columns are reordered on the host so the projection
emits Q_lo = [h0 d0:32 | h1 d0:32 | ...] and Q_hi = [h0 d32:64 | ...] as two
128-partition blocks; rotate_half is then two full-width [128, T] tensor ops
instead of per-head 32-row slices.
"""

import os
import sys
import numpy as np

N_CORES = 8
B, T, C = 2, 2048, 1024
H = 16
HD = 64
HPC = 4            # heads per core
NT = 4             # token tiles of 512
TQ = 512           # tq tile size
KC = C // 128      # contraction chunks for qkv projection

_PROGRAM_CACHE = {}


def _build_program(has_battn: bool, has_bproj: bool, p_bf16: bool,
                   debug_taps: bool = False):
    import concourse.bass as bass
    import concourse.mybir as mybir
    import concourse.bacc as bacc
    import concourse.tile as tile

    F32 = mybir.dt.float32
    F32R = mybir.dt.float32r
    PDT = mybir.dt.bfloat16 if p_bf16 else F32R

    nc = bacc.Bacc("TRN2", target_bir_lowering=False, debug=False,
                   num_devices=N_CORES)

    dbg = {}
    if debug_taps:
        for name, shape in [("dbg_qlo", [128, T]), ("dbg_khi", [128, T]),
                            ("dbg_vaug", [128, 260]), ("dbg_pfull", [128, 512]),
                            ("dbg_pdiag", [128, 512]), ("dbg_av", [65, 512]),
                            ("dbg_rec", [1, 512]), ("dbg_y0", [128, T])]:
            dbg[name] = nc.dram_tensor(name, shape, F32,
                                       kind="ExternalOutput").ap()

    xT = nc.dram_tensor("xT", [C, T], F32R, kind="ExternalInput").ap()
    wqkv = nc.dram_tensor("wqkv", [C, 768], F32R, kind="ExternalInput").ap()
    cos4 = nc.dram_tensor("cos4", [128, T], F32, kind="ExternalInput").ap()
    sin4 = nc.dram_tensor("sin4", [128, T], F32, kind="ExternalInput").ap()
    wp = nc.dram_tensor("wp", [2 * 128, C], F32R, kind="ExternalInput").ap()
    battn = (nc.dram_tensor("battn", [1, 768], F32R, kind="ExternalInput").ap()
             if has_battn else None)
    bproj = (nc.dram_tensor("bproj", [1, C], F32R, kind="ExternalInput").ap()
             if has_bproj else None)
    out = nc.dram_tensor("out", [T, C], F32, kind="ExternalOutput").ap()
    # DRAM scratch for the 1/sumexp partition-broadcast bounce
    rec_d = nc.dram_tensor("rec_d", [16, 512], F32)

    Exp = mybir.ActivationFunctionType.Exp
    scale = 1.0 / float(np.sqrt(HD))

    with tile.TileContext(nc) as tc:
        with (
            tc.tile_pool(name="const", bufs=1) as const,
            tc.tile_pool(name="xp", bufs=3) as xp,
            tc.tile_pool(name="qk", bufs=1) as qkp,
            tc.tile_pool(name="vaug", bufs=1) as vaugp,
            tc.tile_pool(name="tmp", bufs=3) as tmp,
            tc.tile_pool(name="pp", bufs=4) as pp,
            tc.tile_pool(name="rp", bufs=4) as rp,
            tc.tile_pool(name="yrp", bufs=4) as yrp,
            tc.tile_pool(name="yp", bufs=1) as yp,
            tc.tile_pool(name="op", bufs=2) as op,
            tc.tile_pool(name="ps1", bufs=4, space="PSUM") as ps1,
            tc.tile_pool(name="ps2", bufs=2, space="PSUM") as ps2,
        ):
            # ---- constants — interleaved with the first x tile so the
            # first qkv matmul can start after ~2 chunks
            w_sb = const.tile([128, KC, 768], F32R, tag="w")
            xt_view0 = xT.rearrange("(kc p) t -> p kc t", p=128)
            xt0 = xp.tile([128, KC, TQ], F32R, tag="xt", name="xt_0")
            for kc in range(KC):
                nc.sync.dma_start(
                    w_sb[:, kc, :],
                    wqkv.rearrange("(kc p) f -> p kc f", p=128)[:, kc, :])
                nc.sync.dma_start(xt0[:, kc, :], xt_view0[:, kc, 0:TQ])
            cos_sb = const.tile([128, T], F32, tag="cos")
            nc.sync.dma_start(cos_sb[:], cos4[:])
            sin_sb = const.tile([128, T], F32, tag="sin")
            nc.sync.dma_start(sin_sb[:], sin4[:])
            wp_sb = const.tile([128, 2, C], F32R, tag="wp")
            nc.sync.dma_start(wp_sb[:], wp.rearrange("(kb p) f -> p kb f", p=128))
            if has_battn or has_bproj:
                ones_f = const.tile([1, 512], F32, tag="ones_f")
                nc.vector.memset(ones_f[:], 1.0)
                # memset can't write float32r; round-copy from f32 staging
                ones_r = const.tile([1, 512], F32R, tag="ones_r")
                nc.vector.tensor_copy(ones_r[:], ones_f[:])
            if has_battn:
                battn_sb = const.tile([1, 768], F32R, tag="battn")
                nc.sync.dma_start(battn_sb[:], battn[:])
            if has_bproj:
                bproj_sb = const.tile([1, C], F32R, tag="bproj")
                nc.sync.dma_start(bproj_sb[:], bproj[:])

            # persistent activations
            q_lo = qkp.tile([128, T], F32R, tag="qlo")
            q_hi = qkp.tile([128, T], F32R, tag="qhi")
            k_lo = qkp.tile([128, T], F32R, tag="klo")
            k_hi = qkp.tile([128, T], F32R, tag="khi")
            v_aug = vaugp.tile([128, 16, HPC * 65], PDT, tag="vaug")
            y0 = yp.tile([128, T], F32R, tag="y0")
            y1 = yp.tile([128, T], F32R, tag="y1")

            # ones columns of v_aug (col 64 of each head's 65-wide slot)
            for tb in range(16):
                va = v_aug[:, tb, :].rearrange("p (h c) -> p h c", c=65)
                nc.vector.memset(va[:, :, 64:65], 1.0)

            xt_view = xT.rearrange("(kc p) t -> p kc t", p=128)

            # prefetch remaining x tiles so qkv matmuls never wait on DMA
            xts = [xt0]
            for n in range(1, NT):
                tsl = bass.ts(n, TQ)
                xt = xp.tile([128, KC, TQ], F32R, tag="xt", name=f"xt_{n}")
                for kc in range(KC):
                    nc.sync.dma_start(xt[:, kc, :], xt_view[:, kc, tsl])
                xts.append(xt)

            # qkv projection + rope + v staging for token tile n
            def qkv_tile(n):
                tsl = bass.ts(n, TQ)
                xt = xts[n]

                # q_lo|q_hi pair then k_lo|k_hi pair, each in a 2-bank psum
                for (pair, d_lo, d_hi) in ((0, q_lo, q_hi), (1, k_lo, k_hi)):
                    pq = ps2.tile([128, 2 * TQ], F32, tag="ps2",
                                  name=f"qk_{n}_{pair}")
                    for half in range(2):
                        fb = 2 * pair + half
                        dst = pq[:, half * TQ:(half + 1) * TQ]
                        for kc in range(KC):
                            nc.tensor.matmul(
                                dst,
                                w_sb[:, kc, bass.ts(fb, 128)],
                                xt[:, kc, :],
                                start=(kc == 0),
                                stop=(kc == KC - 1 and not has_battn),
                            )
                        if has_battn:
                            nc.tensor.matmul(
                                dst,
                                battn_sb[0:1, bass.ts(fb, 128)],
                                ones_r[0:1, :TQ],
                                start=False, stop=True,
                            )
                    # rope fused into psum eviction:
                    #   out_lo = lo*cos - hi*sin ; out_hi = hi*cos + lo*sin
                    cs = cos_sb[:, tsl]
                    sn = sin_sb[:, tsl]
                    p_lo = pq[:, 0:TQ]
                    p_hi = pq[:, TQ:2 * TQ]
                    t1 = tmp.tile([128, TQ], F32, tag="t")
                    nc.vector.tensor_mul(t1[:], p_lo, cs)
                    t2 = tmp.tile([128, TQ], F32, tag="t")
                    nc.vector.tensor_mul(t2[:], p_hi, sn)
                    nc.vector.tensor_sub(d_lo[:, tsl], t1[:], t2[:])
                    t3 = tmp.tile([128, TQ], F32, tag="t")
                    nc.vector.tensor_mul(t3[:], p_hi, cs)
                    t4 = tmp.tile([128, TQ], F32, tag="t")
                    nc.vector.tensor_mul(t4[:], p_lo, sn)
                    nc.vector.tensor_add(d_hi[:, tsl], t3[:], t4[:])

                # v in [token, feat] layout: lhsT = xT chunk, rhs = w_v
                for tb in range(4):
                    blk = 4 * n + tb
                    pv = ps1.tile([128, 512], F32, tag="ps1",
                                  name=f"pv_{n}_{tb}")
                    for kc in range(KC):
                        nc.tensor.matmul(
                            pv[:, 0:256],
                            xt[:, kc, bass.ts(tb, 128)],
                            w_sb[:, kc, 512:768],
                            start=(kc == 0),
                            stop=(kc == KC - 1 and not has_battn),
                        )
                    if has_battn:
                        nc.tensor.matmul(
                            pv[:, 0:256], ones_r[0:1, 0:128],
                            battn_sb[0:1, 512:768],
                            start=False, stop=True,
                        )
                    va = v_aug[:, blk, :].rearrange("p (h c) -> p h c", c=65)
                    pv_h = pv[:, 0:256].rearrange("p (h d) -> p h d", d=64)
                    nc.vector.tensor_copy(va[:, :, 0:64], pv_h[:])

            # attention + c_proj for query tile i (needs qkv tiles 0..i)
            def attn_tile(i, deferred):
                tq_sl = bass.ts(i, TQ)
                av_ps = [ps1.tile([65, TQ], F32, tag="ps1", name=f"av_{i}_{h}")
                         for h in range(HPC)]
                n_k = 4 * i + 4
                for kb in range(n_k):
                    ksl = bass.ts(kb, 128)
                    j = kb - 4 * i
                    skip = 128 * j if j > 0 else 0   # fully-masked lead cols
                    W_v = TQ - skip                  # valid width per half
                    for pair in range(2):
                        s2 = ps2.tile([128, 2 * TQ], F32, tag="ps2",
                                      name=f"s_{i}_{kb}_{pair}")
                        for half in range(2):
                            h = 2 * pair + half
                            hs = slice(32 * h, 32 * h + 32)
                            dst = s2[:, half * TQ + skip:(half + 1) * TQ]
                            qsl = bass.ds(i * TQ + skip, W_v)
                            nc.tensor.matmul(dst, k_lo[hs, ksl],
                                             q_lo[hs, qsl],
                                             start=True, stop=False,
                                             tile_position=(32 * h, 0))
                            nc.tensor.matmul(dst, k_hi[hs, ksl],
                                             q_hi[hs, qsl],
                                             start=False, stop=True,
                                             tile_position=(32 * h, 0))
                        p_t = pp.tile([128, 2 * TQ], PDT, tag="p")
                        s2_v = s2[:].rearrange("p (g t) -> p g t", g=2)
                        pt_v = p_t[:].rearrange("p (g t) -> p g t", g=2)
                        nc.scalar.activation(pt_v[:, :, skip:TQ],
                                             s2_v[:, :, skip:TQ],
                                             Exp, scale=scale)
                        if skip:
                            nc.vector.memset(pt_v[:, :, 0:skip], 0.0)
                        if j >= 0:
                            # 128-wide causal triangle (keep local col >= row)
                            for half in range(2):
                                off = half * TQ + skip
                                nc.gpsimd.affine_select(
                                    out=p_t[:, off:off + 128],
                                    in_=p_t[:, off:off + 128],
                                    compare_op=mybir.AluOpType.is_ge,
                                    fill=0.0,
                                    base=0,
                                    pattern=[[1, 128]],
                                    channel_multiplier=-1,
                                )
                        if debug_taps and i == 1 and pair == 0 and kb in (2, 5):
                            nm = "dbg_pfull" if kb == 2 else "dbg_pdiag"
                            dp = tmp.tile([128, 512], F32, tag="dbgp",
                                          name=f"dbgp_{kb}")
                            nc.vector.tensor_copy(dp[:], p_t[:, TQ:2 * TQ])
                            nc.sync.dma_start(dbg[nm][:], dp[:])
                        for half in range(2):
                            h = 2 * pair + half
                            nc.tensor.matmul(
                                av_ps[h][:],
                                v_aug[:, kb, bass.ts(h, 65)],
                                p_t[:, half * TQ:(half + 1) * TQ],
                                start=(kb == 0),
                                stop=(kb == n_k - 1),
                            )
                    if kb == 0:
                        for fn in deferred:
                            fn()
                        deferred.clear()

                # evict raw av immediately (frees the psum accumulators),
                # then normalize off the critical path: 1/sum via approx
                # reciprocal, partition-broadcast via a DRAM-bounce DMA
                for h in range(HPC):
                    yr = yrp.tile([65, TQ], F32, tag="yr")
                    nc.any.tensor_copy(yr[:], av_ps[h][:])
                    # reciprocal_approx_fast needs a base-partition-0 SBUF src
                    se = rp.tile([1, TQ], F32, tag="se")
                    nc.any.tensor_copy(se[:], yr[64:65, :])
                    rec = rp.tile([1, TQ], F32, tag="r")
                    nc.vector.reciprocal_approx_fast(rec[:], se[:])
                    if debug_taps and i == 1 and h == 1:
                        nc.sync.dma_start(dbg["dbg_av"][:], yr[:])
                        nc.sync.dma_start(dbg["dbg_rec"][:], rec[:])
                    slot = rec_d[4 * i + h:4 * i + h + 1, :]
                    nc.sync.dma_start(slot, rec[:])
                    bc_sb = tmp.tile([64, TQ], F32, tag="bc")
                    bcast_src = bass.AP(
                        tensor=slot.tensor, offset=slot.offset,
                        ap=[[0, 64], [1, TQ]])
                    nc.sync.dma_start(bc_sb[:], bcast_src)
                    y_t = y0 if h < 2 else y1
                    y_sl = y_t[(h % 2) * 64:(h % 2) * 64 + 64, tq_sl]
                    nc.vector.tensor_mul(y_sl, yr[0:64, :], bc_sb[:])

                # c_proj for the 4 finished token blocks; deferred so it
                # fills PE gaps during the NEXT tile instead of stalling
                def cproj():
                    for m in range(4 * i, 4 * i + 4):
                        msl = bass.ts(m, 128)
                        for n2 in range(2):
                            nsl = bass.ts(n2, 512)
                            po = ps1.tile([128, 512], F32, tag="ps1",
                                          name=f"po_{m}_{n2}")
                            nc.tensor.matmul(po[:], y0[:, msl],
                                             wp_sb[:, 0, nsl],
                                             start=True, stop=False)
                            nc.tensor.matmul(po[:], y1[:, msl],
                                             wp_sb[:, 1, nsl],
                                             start=False, stop=not has_bproj)
                            if has_bproj:
                                nc.tensor.matmul(
                                    po[:], ones_r[0:1, 0:128],
                                    bproj_sb[0:1, nsl],
                                    start=False, stop=True,
                                )
                            o_t = op.tile([128, 512], F32, tag="o")
                            nc.any.tensor_copy(o_t[:], po[:])
                            nc.sync.dma_start(out[msl, nsl], o_t[:])
                return cproj

            # interleave so rope (DVE) overlaps attention (ACT/PE)
            deferred = []
            for n in range(NT):
                qkv_tile(n)
                deferred = [attn_tile(n, deferred)]
            deferred[0]()

            if debug_taps:
                nc.sync.dma_start(dbg["dbg_qlo"][:], q_lo[:].bitcast(F32))
                nc.sync.dma_start(dbg["dbg_khi"][:], k_hi[:].bitcast(F32))
                dv = tmp.tile([128, 260], F32, tag="dbgv")
                nc.vector.tensor_copy(dv[:], v_aug[:, 3, :])
                nc.sync.dma_start(dbg["dbg_vaug"][:], dv[:])
                nc.sync.dma_start(dbg["dbg_y0"][:], y0[:].bitcast(F32))

    nc.finalize()
    return nc


def _get_program(has_battn, has_bproj, p_bf16=True):
    key = (has_battn, has_bproj, p_bf16)
    if key not in _PROGRAM_CACHE:
        _PROGRAM_CACHE[key] = _build_program(*key)
    return _PROGRAM_CACHE[key]


def _rope_tables_np():
    inv_freq = (1.0 / (10000.0 ** (np.arange(0, HD, 2, dtype=np.float32) / HD)))
    t = np.arange(T, dtype=np.float32)
    freqs = np.outer(t, inv_freq).astype(np.float32)      # [T, 32]
    cos4 = np.tile(np.cos(freqs).T, (4, 1))               # [128, T]
    sin4 = np.tile(np.sin(freqs).T, (4, 1))
    return np.ascontiguousarray(cos4), np.ascontiguousarray(sin4)


def _install_trace_shim():
    """Optional: lets run_bass_kernel_spmd(trace=True) capture NTFF profiles."""
    import contextlib
    import ctypes
    import types

    so = "/opt/axon/libaxon_pjrt.so"
    if not os.path.exists(so) or "antenv.axon_hooks" in sys.modules:
        return
    try:
        lib = ctypes.CDLL(so)
        if not hasattr(lib, "axon_start_nrt_profile"):
            return
        lib.axon_start_nrt_profile.argtypes = [ctypes.POINTER(ctypes.c_int64),
                                               ctypes.c_size_t]
        lib.axon_start_nrt_profile.restype = ctypes.c_int64
        lib.axon_stop_nrt_profile.argtypes = [ctypes.c_char_p]
        lib.axon_stop_nrt_profile.restype = ctypes.c_int64

        @contextlib.contextmanager
        def _hook(output_dir, device_ids):
            import jax
            jax.devices()
            if device_ids:
                ids = (ctypes.c_int64 * len(device_ids))(*device_ids)
                rc = lib.axon_start_nrt_profile(ids, len(device_ids))
            else:
                rc = lib.axon_start_nrt_profile(None, 0)
            if rc != 0:
                raise RuntimeError(f"axon_start_nrt_profile rc={rc}")
            try:
                yield
            finally:
                n = lib.axon_stop_nrt_profile(str(output_dir).encode())
                print(f"profile: {n} file(s) written to {output_dir}",
                      file=sys.stderr)

        mod = types.ModuleType("antenv.axon_hooks")
        mod.get_axon_ntff_profile_hook = lambda: _hook
        mod.set_axon_ntff_profile_hook = lambda h: None
        sys.modules["antenv.axon_hooks"] = mod
    except Exception:
        pass


def kernel(x, W_attn, b_attn, W_proj, b_proj):
    from concourse.bass_utils import run_bass_kernel_spmd

    x = np.asarray(x, dtype=np.float32)
    W_attn = np.asarray(W_attn, dtype=np.float32)
    b_attn = np.asarray(b_attn, dtype=np.float32)
    W_proj = np.asarray(W_proj, dtype=np.float32)
    b_proj = np.asarray(b_proj, dtype=np.float32)

    has_battn = bool(np.any(b_attn))
    has_bproj = bool(np.any(b_proj))
    nc = _get_program(has_battn, has_bproj)

    cos4, sin4 = _rope_tables_np()
    dd32 = np.arange(32)
    dd64 = np.arange(64)

    in_maps = []
    for c in range(N_CORES):
        b = c // 4
        g = c % 4
        hs = 4 * g + np.arange(HPC)
        qlo = (hs[:, None] * HD + dd32[None, :]).ravel()
        qhi = qlo + 32
        cols = np.concatenate([qlo, qhi, 1024 + qlo, 1024 + qhi,
                               2048 + (hs[:, None] * HD + dd64[None, :]).ravel()])
        rows = (hs[:, None] * HD + dd64[None, :]).ravel()
        m = {
            "xT": np.ascontiguousarray(x[b].T),
            "wqkv": np.ascontiguousarray(W_attn[:, cols]),
            "cos4": cos4,
            "sin4": sin4,
            "wp": np.ascontiguousarray(W_proj[rows, :]),
        }
        if has_battn:
            m["battn"] = np.ascontiguousarray(b_attn[cols][None, :])
        if has_bproj:
            # only one core per batch group contributes the bias (host sums 4)
            bp = b_proj if g == 0 else np.zeros_like(b_proj)
            m["bproj"] = np.ascontiguousarray(bp[None, :])
        in_maps.append(m)

    trace_dir = os.environ.get("BASSK_TRACE")
    kwargs = {}
    if trace_dir:
        _install_trace_shim()
        kwargs = {"trace": True, "tmpdir": trace_dir,
                  "trace_cores": [0], "stitch_traces": False}

    res = run_bass_kernel_spmd(nc, in_maps, core_ids=list(range(N_CORES)),
                               **kwargs)
    if trace_dir:
        kernel._last_result = res

    out = np.zeros((B, T, C), dtype=np.float32)
    for c in range(N_CORES):
        out[c // 4] += res.results[c]["out"]
    return out
